# Initial kernel scaffold
#
"""BitLinear forward (fake-quant int8 activations x ternary weight) on 8 TRN2 cores.

Strategy (data-parallel, per sharding hint):
  - Shard x over the flattened (B*S) token dim: 8192 rows per core.
  - Replicate the ternary weight (pre-dequantized/transposed to bf16 [D_IN, D_OUT]
    on host -- exact, values in {-1,0,1}) and bias on every core.
  - On device per 128-row tile:
      PE-transpose x[s,i] -> x.T[i,s] (fp32 raw mode),
      ACT drains PSUM applying *1/scale,
      DVE clips to [-127,127], then rounds to nearest-even integer via the
      fp32 magic-number trick and casts bf16 (exact: |int| <= 127),
      PE matmul accumulates over the 8 K-tiles in fp32 PSUM (exact integer
      accumulation: products <= 127, sums < 2^24),
      DVE applies out = psum*scale + bias straight out of PSUM,
      DMA out.
"""

import numpy as np
import ml_dtypes

B, S, D = 16, 4096, 1024
N_CORES = 8
ROWS = (B * S) // N_CORES  # 8192 rows per core
P = 128
NT = ROWS // P             # 64 row tiles per core
KT = D // P                # 8 contraction tiles
QB = 127.0
MAGIC = float(1.5 * 2 ** 23)  # fp32 round-to-nearest-even magic constant

_NC_CACHE = {}


def _build_nc(nt=NT):
    import concourse.bass as bass
    import concourse.mybir as mybir
    from concourse.tile import TileContext
    from concourse.masks import make_identity

    fp32 = mybir.dt.float32
    bf16 = mybir.dt.bfloat16
    Alu = mybir.AluOpType
    Act = mybir.ActivationFunctionType

    nc = bass.Bass()
    rows = nt * P
    x = nc.dram_tensor("x", [rows, D], fp32, kind="ExternalInput")
    # wt: w.T with the K dim folded: wt[p, b*D + o] = (ternary_weight[o, b*128+p] - 1)
    wt = nc.dram_tensor("wt", [P, KT * D], bf16, kind="ExternalInput")
    bias_b = nc.dram_tensor("bias_b", [P, D], fp32, kind="ExternalInput")
    scal = nc.dram_tensor("scal", [P, 2], fp32, kind="ExternalInput")  # [scale, 1/scale]
    out = nc.dram_tensor("out", [rows, D], fp32, kind="ExternalOutput")

    with TileContext(nc) as tc:
        with (
            tc.tile_pool(name="const", bufs=1) as constp,
            tc.tile_pool(name="xin", bufs=4) as xp,
            tc.tile_pool(name="work", bufs=3) as wp,
            tc.tile_pool(name="ptp", bufs=2, space="PSUM") as ptp,
            tc.tile_pool(name="pop", bufs=2, space="PSUM") as pop,
            tc.tile_pool(name="oout", bufs=4) as op_,
        ):
            ident = constp.tile([P, P], fp32)
            make_identity(nc, ident)
            wt_sb = constp.tile([P, KT * D], bf16)
            nc.sync.dma_start(out=wt_sb, in_=wt[:, :])
            bias_sb = constp.tile([P, D], fp32)
            nc.sync.dma_start(out=bias_sb, in_=bias_b[:, :])
            sc = constp.tile([P, 2], fp32)
            nc.sync.dma_start(out=sc, in_=scal[:, :])

            for st in range(nt):
                xa = xp.tile([P, D], fp32, name="xa")
                nc.sync.dma_start(out=xa, in_=x[st * P:(st + 1) * P, :])

                # x[s, i] -> x.T[i, s] in 128x128 chunks (PE raw transpose)
                pt = ptp.tile([P, D], fp32, name="pt")
                for b in range(KT):
                    nc.tensor.transpose(
                        pt[:, b * P:(b + 1) * P], xa[:, b * P:(b + 1) * P], ident
                    )

                # t = x.T * (1/scale)   (ACT drains PSUM with free affine)
                tt = wp.tile([P, D], fp32, name="tt")
                nc.scalar.activation(tt, pt, Act.Copy, scale=sc[:, 1:2])

                # clip to [-127, 127]
                uu = wp.tile([P, D], fp32, name="uu")
                nc.vector.tensor_scalar(uu, tt, -QB, QB, Alu.max, Alu.min)

                # round to nearest(-even) integer; cast to bf16 (exact for |v|<=127)
                qq = wp.tile([P, D], bf16, name="qq")
                nc.vector.tensor_scalar(qq, uu, MAGIC, MAGIC, Alu.add, Alu.subtract)

                # psum[s, o] = sum_i q.T[i, s] * wt[i, o]
                po = pop.tile([P, D], fp32, name="po")
                for b in range(KT):
                    first = b == 0
                    last = b == KT - 1
                    qs = qq[:, b * P:(b + 1) * P]
                    nc.tensor.matmul(
                        po[:, 0:512], qs, wt_sb[:, b * D:b * D + 512],
                        start=first, stop=last,
                    )
                    nc.tensor.matmul(
                        po[:, 512:1024], qs, wt_sb[:, b * D + 512:(b + 1) * D],
                        start=first, stop=last,
                    )

                # out = psum * scale + bias
                oo = op_.tile([P, D], fp32, name="oo")
                nc.vector.scalar_tensor_tensor(
                    oo, po, sc[:, 0:1], bias_sb, Alu.mult, Alu.add
                )
                nc.scalar.dma_start(out=out[st * P:(st + 1) * P, :], in_=oo)
    return nc


def _get_nc(nt=NT):
    if nt not in _NC_CACHE:
        _NC_CACHE[nt] = _build_nc(nt)
    return _NC_CACHE[nt]


def _prep_inputs(x, ternary_weight, bias, act_scale, n_cores=N_CORES, rows=ROWS):
    x = np.asarray(x, dtype=np.float32)
    tw = np.asarray(ternary_weight)
    bias = np.asarray(bias, dtype=np.float32)

    scale = np.maximum(np.float32(act_scale), np.float32(1e-5))
    inv = np.float32(1.0) / scale

    # w.T [i, o] = tw[o, i] - 1, exact in bf16; fold to [128, KT*D] so the
    # device-side SBUF tile is one contiguous DMA.
    wt = (tw.T.astype(np.float32) - 1.0).astype(ml_dtypes.bfloat16)  # [D_IN, D_OUT]
    wt_folded = np.ascontiguousarray(
        wt.reshape(KT, P, D).transpose(1, 0, 2).reshape(P, KT * D)
    )
    bias_b = np.ascontiguousarray(np.broadcast_to(bias[None, :], (P, D)))
    scal = np.ascontiguousarray(
        np.broadcast_to(np.array([scale, inv], dtype=np.float32)[None, :], (P, 2))
    )

    xf = x.reshape(-1, D)
    in_maps = []
    for c in range(n_cores):
        in_maps.append({
            "x": np.ascontiguousarray(xf[c * rows:(c + 1) * rows]),
            "wt": wt_folded,
            "bias_b": bias_b,
            "scal": scal,
        })
    return in_maps


def kernel(x, ternary_weight, bias, act_scale):
    from concourse.bass_utils import run_bass_kernel_spmd

    in_maps = _prep_inputs(x, ternary_weight, bias, act_scale)
    nc = _get_nc()
    res = run_bass_kernel_spmd(nc, in_maps, core_ids=list(range(N_CORES)))
    out = np.concatenate([r["out"] for r in res.results], axis=0)
    return out.reshape(B, S, D)


# revision 6
# speedup vs baseline: 4.1088x; 4.1088x over previous
"""BitLinear forward (fake-quant int8 activations x ternary weight) on 8 TRN2 cores.

Strategy (data-parallel, per sharding hint):
  - Shard x over the flattened (B*S) token dim: 8192 rows per core.
  - Replicate the ternary weight (pre-dequantized/transposed to bf16 [D_IN, D_OUT]
    on host -- exact, values in {-1,0,1}) and bias on every core.
  - On device per 128-row tile:
      PE-transpose x[s,i] -> x.T[i,s] (fp32 raw mode),
      ACT drains PSUM applying *1/scale,
      DVE clips to [-127,127], then rounds to nearest-even integer via the
      fp32 magic-number trick and casts bf16 (exact: |int| <= 127),
      PE matmul accumulates over the 8 K-tiles in fp32 PSUM (exact integer
      accumulation: products <= 127, sums < 2^24),
      DVE applies out = psum*scale + bias straight out of PSUM,
      DMA out.
"""

import numpy as np
import ml_dtypes

B, S, D = 16, 4096, 1024
N_CORES = 8
ROWS = (B * S) // N_CORES  # 8192 rows per core
P = 128
NT = ROWS // P             # 64 row tiles per core
KT = D // P                # 8 contraction tiles
QB = 127.0
MAGIC = float(1.5 * 2 ** 23)  # fp32 round-to-nearest-even magic constant

_NC_CACHE = {}


def _build_nc(nt=NT, repeat=1):
    import concourse.mybir as mybir
    from concourse import bacc
    from concourse.tile import TileContext
    from concourse.masks import make_identity

    fp32 = mybir.dt.float32
    bf16 = mybir.dt.bfloat16
    Alu = mybir.AluOpType
    Act = mybir.ActivationFunctionType

    nc = bacc.Bacc(None, target_bir_lowering=False)
    rows = nt * P
    x = nc.dram_tensor("x", [rows, D], fp32, kind="ExternalInput")
    # wt: w.T with the K dim folded: wt[p, b*D + o] = (ternary_weight[o, b*128+p] - 1)
    wt = nc.dram_tensor("wt", [P, KT * D], bf16, kind="ExternalInput")
    bias_b = nc.dram_tensor("bias_b", [P, D], fp32, kind="ExternalInput")
    scal = nc.dram_tensor("scal", [P, 2], fp32, kind="ExternalInput")  # [scale, 1/scale]
    out = nc.dram_tensor("out", [rows, D], fp32, kind="ExternalOutput")

    with TileContext(nc) as tc:
        with (
            tc.tile_pool(name="const", bufs=1) as constp,
            tc.tile_pool(name="xin", bufs=4) as xp,
            tc.tile_pool(name="work", bufs=3) as wp,
            tc.tile_pool(name="ptp", bufs=2, space="PSUM") as ptp,
            tc.tile_pool(name="pop", bufs=2, space="PSUM") as pop,
            tc.tile_pool(name="oout", bufs=4) as op_,
        ):
            ident = constp.tile([P, P], fp32)
            make_identity(nc, ident)
            wt_sb = constp.tile([P, KT * D], bf16)
            nc.sync.dma_start(out=wt_sb, in_=wt[:, :])
            bias_sb = constp.tile([P, D], fp32)
            nc.sync.dma_start(out=bias_sb, in_=bias_b[:, :])
            sc = constp.tile([P, 2], fp32)
            nc.sync.dma_start(out=sc, in_=scal[:, :])

            for st in [t for _ in range(repeat) for t in range(nt)]:
                xa = xp.tile([P, D], fp32, name="xa")
                nc.sync.dma_start(out=xa, in_=x[st * P:(st + 1) * P, :])

                # x[s, i] -> x.T[i, s] in 128x128 chunks (PE raw transpose)
                pt = ptp.tile([P, D], fp32, name="pt")
                for b in range(KT):
                    nc.tensor.transpose(
                        pt[:, b * P:(b + 1) * P], xa[:, b * P:(b + 1) * P], ident
                    )

                # t = x.T * (1/scale)   (ACT drains PSUM with free affine)
                tt = wp.tile([P, D], fp32, name="tt")
                nc.scalar.activation(tt, pt, Act.Copy, scale=sc[:, 1:2])

                # clip to [-127, 127]
                uu = wp.tile([P, D], fp32, name="uu")
                nc.vector.tensor_scalar(uu, tt, -QB, QB, Alu.max, Alu.min)

                # round to nearest(-even) integer; cast to bf16 (exact for |v|<=127)
                qq = wp.tile([P, D], bf16, name="qq")
                nc.vector.tensor_scalar(qq, uu, MAGIC, MAGIC, Alu.add, Alu.subtract)

                # psum[s, o] = sum_i q.T[i, s] * wt[i, o]
                po = pop.tile([P, D], fp32, name="po")
                for b in range(KT):
                    first = b == 0
                    last = b == KT - 1
                    qs = qq[:, b * P:(b + 1) * P]
                    nc.tensor.matmul(
                        po[:, 0:512], qs, wt_sb[:, b * D:b * D + 512],
                        start=first, stop=last,
                    )
                    nc.tensor.matmul(
                        po[:, 512:1024], qs, wt_sb[:, b * D + 512:(b + 1) * D],
                        start=first, stop=last,
                    )

                # out = psum * scale + bias
                oo = op_.tile([P, D], fp32, name="oo")
                nc.vector.scalar_tensor_tensor(
                    oo, po, sc[:, 0:1], bias_sb, Alu.mult, Alu.add
                )
                nc.scalar.dma_start(out=out[st * P:(st + 1) * P, :], in_=oo)
    nc.compile()
    return nc


def _get_nc(nt=NT):
    if nt not in _NC_CACHE:
        _NC_CACHE[nt] = _build_nc(nt)
    return _NC_CACHE[nt]


def _prep_inputs(x, ternary_weight, bias, act_scale, n_cores=N_CORES, rows=ROWS):
    x = np.asarray(x, dtype=np.float32)
    tw = np.asarray(ternary_weight)
    bias = np.asarray(bias, dtype=np.float32)

    scale = np.maximum(np.float32(act_scale), np.float32(1e-5))
    inv = np.float32(1.0) / scale

    # w.T [i, o] = tw[o, i] - 1, exact in bf16; fold to [128, KT*D] so the
    # device-side SBUF tile is one contiguous DMA.
    wt = (tw.T.astype(np.float32) - 1.0).astype(ml_dtypes.bfloat16)  # [D_IN, D_OUT]
    wt_folded = np.ascontiguousarray(
        wt.reshape(KT, P, D).transpose(1, 0, 2).reshape(P, KT * D)
    )
    bias_b = np.ascontiguousarray(np.broadcast_to(bias[None, :], (P, D)))
    scal = np.ascontiguousarray(
        np.broadcast_to(np.array([scale, inv], dtype=np.float32)[None, :], (P, 2))
    )

    xf = x.reshape(-1, D)
    in_maps = []
    for c in range(n_cores):
        in_maps.append({
            "x": np.ascontiguousarray(xf[c * rows:(c + 1) * rows]),
            "wt": wt_folded,
            "bias_b": bias_b,
            "scal": scal,
        })
    return in_maps


def kernel(x, ternary_weight, bias, act_scale):
    from concourse.bass_utils import run_bass_kernel_spmd

    in_maps = _prep_inputs(x, ternary_weight, bias, act_scale)
    nc = _get_nc()
    res = run_bass_kernel_spmd(nc, in_maps, core_ids=list(range(N_CORES)))
    out = np.concatenate([r["out"] for r in res.results], axis=0)
    return out.reshape(B, S, D)


# revision 10
# speedup vs baseline: 12.4754x; 3.0363x over previous
"""BitLinear forward (fake-quant int8 activations x ternary weight) on 8 TRN2 cores.

Strategy (data-parallel, per sharding hint):
  - Shard x over the flattened (B*S) token dim: 8192 rows per core.
  - Replicate the ternary weight (pre-dequantized/transposed to bf16 [D_IN, D_OUT]
    on host -- exact, values in {-1,0,1}) and bias on every core.
  - On device per 128-row tile:
      PE-transpose x[s,i] -> x.T[i,s] (fp32 raw mode),
      ACT drains PSUM applying *1/scale,
      DVE clips to [-127,127], then rounds to nearest-even integer via the
      fp32 magic-number trick and casts bf16 (exact: |int| <= 127),
      PE matmul accumulates over the 8 K-tiles in fp32 PSUM (exact integer
      accumulation: products <= 127, sums < 2^24),
      DVE applies out = psum*scale + bias straight out of PSUM,
      DMA out.
"""

import numpy as np
import ml_dtypes

B, S, D = 16, 4096, 1024
N_CORES = 8
ROWS = (B * S) // N_CORES  # 8192 rows per core
P = 128
NT = ROWS // P             # 64 row tiles per core
KT = D // P                # 8 contraction tiles
QB = 127.0
MAGIC = float(1.5 * 2 ** 23)  # fp32 round-to-nearest-even magic constant

_NC_CACHE = {}


def _build_nc(nt=NT, repeat=1, xin_bufs=4, work_bufs=3, out_bufs=4,
              pt_bufs=2, po_bufs=2, out_dma_engine="scalar"):
    import concourse.mybir as mybir
    from concourse import bacc
    from concourse.tile import TileContext
    from concourse.masks import make_identity

    fp32 = mybir.dt.float32
    bf16 = mybir.dt.bfloat16
    Alu = mybir.AluOpType
    Act = mybir.ActivationFunctionType

    nc = bacc.Bacc(None, target_bir_lowering=False)
    rows = nt * P
    x = nc.dram_tensor("x", [rows, D], fp32, kind="ExternalInput")
    # wt: w.T with the K dim folded: wt[p, b*D + o] = (ternary_weight[o, b*128+p] - 1)
    wt = nc.dram_tensor("wt", [P, KT * D], bf16, kind="ExternalInput")
    bias_b = nc.dram_tensor("bias_b", [P, D], fp32, kind="ExternalInput")
    scal = nc.dram_tensor("scal", [P, 2], fp32, kind="ExternalInput")  # [scale, 1/scale]
    out = nc.dram_tensor("out", [rows, D], fp32, kind="ExternalOutput")

    with TileContext(nc) as tc:
        with (
            tc.tile_pool(name="const", bufs=1) as constp,
            tc.tile_pool(name="xin", bufs=xin_bufs) as xp,
            tc.tile_pool(name="work", bufs=work_bufs) as wp,
            tc.tile_pool(name="ptp", bufs=pt_bufs, space="PSUM") as ptp,
            tc.tile_pool(name="pop", bufs=po_bufs, space="PSUM") as pop,
            tc.tile_pool(name="oout", bufs=out_bufs) as op_,
        ):
            ident = constp.tile([P, P], fp32)
            make_identity(nc, ident)
            wt_sb = constp.tile([P, KT * D], bf16)
            nc.sync.dma_start(out=wt_sb, in_=wt[:, :])
            bias_sb = constp.tile([P, D], fp32)
            nc.sync.dma_start(out=bias_sb, in_=bias_b[:, :])
            sc = constp.tile([P, 2], fp32)
            nc.sync.dma_start(out=sc, in_=scal[:, :])

            for st in [t for _ in range(repeat) for t in range(nt)]:
                xa = xp.tile([P, D], fp32, name="xa")
                nc.sync.dma_start(out=xa, in_=x[st * P:(st + 1) * P, :])

                # x[s, i] -> x.T[i, s] in 128x128 chunks (PE raw transpose)
                pt = ptp.tile([P, D], fp32, name="pt")
                for b in range(KT):
                    nc.tensor.transpose(
                        pt[:, b * P:(b + 1) * P], xa[:, b * P:(b + 1) * P], ident
                    )

                # t = x.T * (1/scale)   (ACT drains PSUM with free affine)
                tt = wp.tile([P, D], fp32, name="tt")
                nc.scalar.activation(tt, pt, Act.Copy, scale=sc[:, 1:2])

                # clip to [-127, 127]
                uu = wp.tile([P, D], fp32, name="uu")
                nc.vector.tensor_scalar(uu, tt, -QB, QB, Alu.max, Alu.min)

                # round to nearest(-even) integer; cast to bf16 (exact for |v|<=127)
                qq = wp.tile([P, D], bf16, name="qq")
                nc.vector.tensor_scalar(qq, uu, MAGIC, MAGIC, Alu.add, Alu.subtract)

                # psum[s, o] = sum_i q.T[i, s] * wt[i, o]
                po = pop.tile([P, D], fp32, name="po")
                for b in range(KT):
                    first = b == 0
                    last = b == KT - 1
                    qs = qq[:, b * P:(b + 1) * P]
                    nc.tensor.matmul(
                        po[:, 0:512], qs, wt_sb[:, b * D:b * D + 512],
                        start=first, stop=last,
                    )
                    nc.tensor.matmul(
                        po[:, 512:1024], qs, wt_sb[:, b * D + 512:(b + 1) * D],
                        start=first, stop=last,
                    )

                # out = psum * scale + bias
                oo = op_.tile([P, D], fp32, name="oo")
                nc.vector.scalar_tensor_tensor(
                    oo, po, sc[:, 0:1], bias_sb, Alu.mult, Alu.add
                )
                out_eng = getattr(nc, out_dma_engine)
                out_eng.dma_start(out=out[st * P:(st + 1) * P, :], in_=oo)
    nc.compile()
    return nc


def _get_nc(nt=NT):
    if nt not in _NC_CACHE:
        _NC_CACHE[nt] = _build_nc(nt)
    return _NC_CACHE[nt]


def _prep_inputs(x, ternary_weight, bias, act_scale, n_cores=N_CORES, rows=ROWS):
    x = np.asarray(x, dtype=np.float32)
    tw = np.asarray(ternary_weight)
    bias = np.asarray(bias, dtype=np.float32)

    scale = np.maximum(np.float32(act_scale), np.float32(1e-5))
    inv = np.float32(1.0) / scale

    # w.T [i, o] = tw[o, i] - 1, exact in bf16; fold to [128, KT*D] so the
    # device-side SBUF tile is one contiguous DMA.
    wt = (tw.T.astype(np.float32) - 1.0).astype(ml_dtypes.bfloat16)  # [D_IN, D_OUT]
    wt_folded = np.ascontiguousarray(
        wt.reshape(KT, P, D).transpose(1, 0, 2).reshape(P, KT * D)
    )
    bias_b = np.ascontiguousarray(np.broadcast_to(bias[None, :], (P, D)))
    scal = np.ascontiguousarray(
        np.broadcast_to(np.array([scale, inv], dtype=np.float32)[None, :], (P, 2))
    )

    xf = x.reshape(-1, D)
    in_maps = []
    for c in range(n_cores):
        in_maps.append({
            "x": np.ascontiguousarray(xf[c * rows:(c + 1) * rows]),
            "wt": wt_folded,
            "bias_b": bias_b,
            "scal": scal,
        })
    return in_maps


def kernel(x, ternary_weight, bias, act_scale):
    from concourse.bass_utils import run_bass_kernel_spmd

    in_maps = _prep_inputs(x, ternary_weight, bias, act_scale)
    nc = _get_nc()
    res = run_bass_kernel_spmd(nc, in_maps, core_ids=list(range(N_CORES)))
    out = np.concatenate([r["out"] for r in res.results], axis=0)
    return out.reshape(B, S, D)
